# revision 7
# baseline (speedup 1.0000x reference)
"""Trainium2 Bass kernel v6 for nn_MultiHeadedAttention_6416681140387.

Two-branch windowed video attention, 8 cores = (video) x (frame).
The host ships x pre-gathered into per-branch window-major layouts
(bf16), with the core's own frame first:
    xw{b}[j, cb, c, ci*ntf + tok]   ci = wy*psz + wx, tok = oh*ohb + ow
Since the QKV convs are 1x1, they consume the window-major layout
directly and produce window-major K/Q with plain contiguous PSUM
evacuations - no strided gathers on device.

Pipeline:
  A: per frame: K conv (+Q conv on frame 0) -> S^T tiles ([k-part, q])
     -> exp (no max-subtraction) -> P^T bf16; row-sums l via ones
     matmuls accumulated across tiles in PSUM; 1/l broadcast via
     rank-1 matmul.
  B/C (per branch): P^T *= bcast(1/l); per frame: V built token-major
     ([tok, c]) from window-major x tiles; PV accumulated per
     frame-pair in PSUM with bias bv as a rank-1 matmul; evacuated
     contiguously into a window-major att_w; final window->pixel
     reorder via identity matmuls (PE reads strided APs at full rate)
     into the 98x98 zero-padded att image.
  D: 3x3 conv + LeakyReLU from the two att images.

Frame tails (576=4*128+64, 144=128+16) are combined into full k-tiles
via small tail-staging buffers so every matmul is M=128 (one M=64).
"""

import sys

if "/opt/trn_rl_repo" not in sys.path:
    sys.path.insert(0, "/opt/trn_rl_repo")

import math
from contextlib import ExitStack

import numpy as np

import concourse.bass as bass
import concourse.tile as tile
from concourse import bacc, mybir
from concourse.masks import make_identity

F32 = mybir.dt.float32
BF16 = mybir.dt.bfloat16

T = 4
C = 256
H = W = 96
PIX = H * W
NCORES = 8

PSZ = [4, 8]
OHB = [24, 12]                 # token grid side
NTF = [576, 144]               # tokens per frame
NCH = [16, 64]                 # feature chunks (psz^2)
NKT = [18, 5]                  # k tiles per video (incl. combined tails)
TAILSZ = [64, 16]              # leftover tokens per frame
SC = [1.0 / math.sqrt(2048.0), 1.0 / math.sqrt(8192.0)]
HALVES = [[(0, 288), (288, 288)], [(0, 144)]]

Exp = mybir.ActivationFunctionType.Exp
Identity = mybir.ActivationFunctionType.Identity


def build(nc):
    xw_d = [nc.dram_tensor(f"xw{b}", [T, C, PIX], BF16, kind="ExternalInput")
            for b in range(2)]
    wqt = nc.dram_tensor("wqt", [C, C], BF16, kind="ExternalInput")
    wkt = nc.dram_tensor("wkt", [C, C], BF16, kind="ExternalInput")
    wvt = nc.dram_tensor("wvt", [C, C], BF16, kind="ExternalInput")
    wot = nc.dram_tensor("wot", [9, C, C], BF16, kind="ExternalInput")
    bq = nc.dram_tensor("bq", [C], F32, kind="ExternalInput")
    bk = nc.dram_tensor("bk", [C], F32, kind="ExternalInput")
    bv = nc.dram_tensor("bv", [C], F32, kind="ExternalInput")
    bo = nc.dram_tensor("bo", [C], F32, kind="ExternalInput")
    out = nc.dram_tensor("out", [C, PIX], F32, kind="ExternalOutput")

    alt = [0]

    def evac(dst, src, bias_ap=None):
        """PSUM -> SBUF evacuation, alternating ACT/DVE."""
        alt[0] ^= 1
        if bias_ap is not None:
            if alt[0]:
                nc.scalar.activation(out=dst, in_=src, func=Identity,
                                     bias=bias_ap, scale=1.0)
            else:
                nc.vector.tensor_scalar_add(dst, src, bias_ap)
        else:
            if alt[0]:
                nc.scalar.copy(dst, src)
            else:
                nc.vector.tensor_copy(dst, src)

    def ap_of(t, off, dims):
        return bass.AP(tensor=t.tensor, offset=t.offset + off,
                       ap=[t.ap[0]] + dims)

    with tile.TileContext(nc, pool_alloc_mode="queue") as tc, ExitStack() as top:
        persist = top.enter_context(tc.tile_pool(name="persist", bufs=1))

        # ---- weights / biases / constants ----
        w_sb = {}
        for name, dt_ in (("wq", wqt), ("wk", wkt), ("wv", wvt)):
            for cb in range(2):
                t = persist.tile([128, C], BF16, name=f"{name}{cb}",
                                 tag=f"{name}{cb}")
                nc.sync.dma_start(out=t,
                                   in_=dt_.ap()[cb * 128:(cb + 1) * 128, :])
                w_sb[(name, cb)] = t

        def bias_tile(name, dt_):
            t = persist.tile([128, 2], F32, name=name, tag=name)
            nc.sync.dma_start(
                out=t, in_=bass.AP(tensor=dt_.ap().tensor, offset=0,
                                   ap=[[1, 128], [128, 2]]))
            return t

        bq_sb = bias_tile("bq", bq)
        bk_sb = bias_tile("bk", bk)
        bo_sb = bias_tile("bo", bo)
        bv_sb = bias_tile("bv", bv)
        ones_col = persist.tile([128, 1], BF16, name="ones_col",
                                tag="ones_col")
        nc.vector.memset(ones_col, 1.0)
        ones_row = persist.tile([1, 288], BF16, name="ones_row",
                                tag="ones_row")
        nc.vector.memset(ones_row, 1.0)
        ident = persist.tile([128, 128], BF16, name="ident", tag="ident")
        make_identity(nc, ident)

        # ---- persistent P^T tiles and broadcast-normalizer ----
        p_pt1 = top.enter_context(tc.tile_pool(name="pt1", bufs=1))
        p_aw = top.enter_context(tc.tile_pool(name="aw", bufs=1))
        es_pt0 = ExitStack()
        p_pt0 = es_pt0.enter_context(tc.tile_pool(name="pt0", bufs=1))
        pt = [[p_pt0.tile([128, 576], BF16, name=f"pt0_{g}", tag=f"pt0_{g}")
               for g in range(NKT[0])],
              [p_pt1.tile([128, 144], BF16, name=f"pt1_{g}", tag=f"pt1_{g}")
               for g in range(NKT[1])]]
        bc = [persist.tile([128, 576], BF16, name="bc0", tag="bc0"),
              persist.tile([128, 144], BF16, name="bc1", tag="bc1")]

        def gidx0(j, t):
            return j * 4 + t

        ecnt = {}

        def l_mm(b, g, rows, l_ps):
            for h, (q0, qn) in enumerate(HALVES[b]):
                k = ecnt.get((b, h), 0)
                nc.tensor.matmul(
                    l_ps[b][h][0:1, :], ones_col[:rows, :],
                    pt[b][g][:rows, q0:q0 + qn],
                    start=(k == 0), stop=(k == NKT[b] - 1))
                ecnt[(b, h)] = k + 1

        # ================= PHASE A: convs + S^T + exp + l =================
        esA = ExitStack()
        p_xw = esA.enter_context(tc.tile_pool(name="xwa", bufs=1))
        p_kw = esA.enter_context(tc.tile_pool(name="kw", bufs=1))
        p_qw = esA.enter_context(tc.tile_pool(name="qw", bufs=1))
        p_ktail = esA.enter_context(tc.tile_pool(name="ktail", bufs=1))
        p_kps = esA.enter_context(tc.tile_pool(name="kps", bufs=3,
                                               space="PSUM"))
        p_sps = esA.enter_context(tc.tile_pool(name="sps", bufs=2,
                                               space="PSUM"))
        p_lps = esA.enter_context(tc.tile_pool(name="lps", bufs=1,
                                               space="PSUM"))
        l_ps = [[p_lps.tile([128, qn], F32, name=f"l{b}_{h}",
                            tag=f"l{b}_{h}")
                 for h, (q0, qn) in enumerate(HALVES[b])] for b in range(2)]
        ktail = [p_ktail.tile([128, NCH[0] * 128], BF16, name="ktail0",
                              tag="ktail0"),
                 p_ktail.tile([128, NCH[1] * 64], BF16, name="ktail1",
                              tag="ktail1")]
        qw = [p_qw.tile([128, 9216], BF16, name=f"qw{b}", tag=f"qw{b}")
              for b in range(2)]

        def conv_half(xpair, w_name, bias_sb, dst, cbo):
            """One branch half: dst [128, 9216] window-major."""
            for ch in range(18):
                ps = p_kps.tile([128, 512], F32, name="kps", tag="kps")
                for cbi in range(2):
                    nc.tensor.matmul(
                        ps,
                        w_sb[(w_name, cbi)][:, cbo * 128:(cbo + 1) * 128],
                        xpair[cbi][:, ch * 512:(ch + 1) * 512],
                        start=(cbi == 0), stop=(cbi == 1))
                evac(dst[:, ch * 512:(ch + 1) * 512], ps,
                     bias_sb[:, cbo:cbo + 1])

        for j in range(T):
            kwf = [p_kw.tile([128, 9216], BF16, name=f"kw{b}", tag=f"kw{b}")
                   for b in range(2)]
            # per branch: load that branch's window layout, conv its half
            for b in range(2):
                xpair = [p_xw.tile([128, PIX], BF16, name=f"xa{cb}",
                                   tag=f"xa{cb}") for cb in range(2)]
                for cb in range(2):
                    for hh in range(2):
                        nc.sync.dma_start(
                            out=xpair[cb][:, hh * 4608:(hh + 1) * 4608],
                            in_=xw_d[b].ap()[j, cb * 128:(cb + 1) * 128,
                                             hh * 4608:(hh + 1) * 4608])
                conv_half(xpair, "wk", bk_sb, kwf[b], b)
                if j == 0:
                    conv_half(xpair, "wq", bq_sb, qw[b], b)

            # save K tails ([ci][par][tsz] layout -> contiguous tail lhsT)
            for b in range(2):
                ntf, nch, tsz = NTF[b], NCH[b], TAILSZ[b]
                npar = 2 if b == 0 else 4
                par = j % npar
                src = kwf[b].rearrange("p (ci tok) -> p ci tok",
                                       ci=nch)[:, :, ntf - tsz:ntf]
                dst = ktail[b].rearrange("p (ci par tok) -> p ci par tok",
                                         ci=nch, par=npar)[:, :, par]
                nc.vector.tensor_copy(dst, src)

            # S^T for this frame's full tiles
            for b in range(2):
                ntf, nch = NTF[b], NCH[b]
                nfull = 4 if b == 0 else 1
                for t in range(nfull):
                    g = gidx0(j, t) if b == 0 else j
                    for h, (q0, qn) in enumerate(HALVES[b]):
                        ps = p_sps.tile([128, 288], F32, name="sps",
                                        tag="sps")
                        for ci in range(nch):
                            nc.tensor.matmul(
                                ps[:, :qn],
                                kwf[b][:, ci * ntf + t * 128:
                                       ci * ntf + t * 128 + 128],
                                qw[b][:, ci * ntf + q0:ci * ntf + q0 + qn],
                                start=(ci == 0), stop=(ci == nch - 1))
                        nc.scalar.activation(
                            out=pt[b][g][:, q0:q0 + qn], in_=ps[:, :qn],
                            func=Exp, scale=SC[b])
                    l_mm(b, g, 128, l_ps)

            if j in (1, 3):
                g = 16 + j // 2
                for h, (q0, qn) in enumerate(HALVES[0]):
                    ps = p_sps.tile([128, 288], F32, name="sps", tag="sps")
                    for ci in range(16):
                        nc.tensor.matmul(
                            ps[:, :qn], ktail[0][:, ci * 128:(ci + 1) * 128],
                            qw[0][:, ci * 576 + q0:ci * 576 + q0 + qn],
                            start=(ci == 0), stop=(ci == 15))
                    nc.scalar.activation(
                        out=pt[0][g][:, q0:q0 + qn], in_=ps[:, :qn],
                        func=Exp, scale=SC[0])
                l_mm(0, g, 128, l_ps)
            if j == 3:
                g = 4
                ps = p_sps.tile([128, 288], F32, name="sps", tag="sps")
                for ci in range(64):
                    nc.tensor.matmul(
                        ps[:64, :144], ktail[1][:, ci * 64:(ci + 1) * 64],
                        qw[1][:, ci * 144:ci * 144 + 144],
                        start=(ci == 0), stop=(ci == 63))
                nc.scalar.activation(out=pt[1][g][:64, :], in_=ps[:64, :144],
                                     func=Exp, scale=SC[1])
                l_mm(1, g, 64, l_ps)

        # 1/l and broadcast rows
        rl = persist.tile([1, 576 + 144], BF16, name="rl", tag="rl")
        o = 0
        with nc.allow_low_precision(reason="1/l in bf16; 2e-2 tolerance"):
            for b in range(2):
                for h, (q0, qn) in enumerate(HALVES[b]):
                    nc.vector.reciprocal(rl[0:1, o:o + qn],
                                         l_ps[b][h][0:1, :])
                    o += qn
        o = 0
        for b in range(2):
            for h, (q0, qn) in enumerate(HALVES[b]):
                ps = p_sps.tile([128, 288], F32, name="sps", tag="sps")
                nc.tensor.matmul(ps[:, :qn], ones_row[0:1, :128],
                                 rl[0:1, o:o + qn], start=True, stop=True)
                evac(bc[b][:, q0:q0 + qn], ps[:, :qn])
                o += qn
        esA.close()

        def att_border_zero(a):
            av = a.rearrange("p (h w) -> p h w", h=98)
            nc.gpsimd.memset(a[:, 0:98], 0.0)
            nc.gpsimd.memset(a[:, 97 * 98:98 * 98], 0.0)
            nc.gpsimd.memset(av[:, 1:97, 0:1], 0.0)
            nc.gpsimd.memset(av[:, 1:97, 97:98], 0.0)

        # ================= PHASES B/C: V build + PV per branch ============
        att_ws = []
        for b in range(2):
            ntf, nch, tsz, psz = NTF[b], NCH[b], TAILSZ[b], PSZ[b]
            ohb = OHB[b]
            for g in range(NKT[b]):
                rows = 64 if (b == 1 and g == 4) else 128
                nc.vector.tensor_mul(pt[b][g][:rows, :], pt[b][g][:rows, :],
                                     bc[b][:rows, :])

            esB = ExitStack()
            p_xb = esB.enter_context(tc.tile_pool(name=f"xb{b}", bufs=2))
            p_xwt = esB.enter_context(tc.tile_pool(name=f"xwt{b}", bufs=1))
            nvb = 9 if b == 0 else 3
            p_v = esB.enter_context(tc.tile_pool(name=f"v{b}", bufs=nvb))
            p_vps = esB.enter_context(tc.tile_pool(name=f"vps{b}", bufs=2,
                                                   space="PSUM"))
            p_pvps = esB.enter_context(tc.tile_pool(name=f"pvps{b}", bufs=4,
                                                    space="PSUM"))
            npar = 2 if b == 0 else 4
            xwtail = [p_xwt.tile([128, nch * npar * tsz], BF16,
                                 name=f"xwt{cb}", tag=f"xwt{cb}")
                      for cb in range(2)]
            att_w = p_aw.tile([128, 9216], BF16, name=f"aw{b}", tag=f"aw{b}")
            att_ws.append(att_w)
            vt = {}

            def vbuild(g, lhsT_of, rows=128):
                v = p_v.tile([128, nch * 128], BF16, name=f"v{b}",
                             tag=f"v{b}")
                vt[g] = v
                for cig in range(nch // 8):
                    ps = p_vps.tile([128, 1024], F32, name=f"vps{b}",
                                    tag=f"vps{b}")
                    for cio in range(8):
                        ci = cig * 8 + cio
                        for cb in range(2):
                            nc.tensor.matmul(
                                ps[:rows, cio * 128:(cio + 1) * 128],
                                lhsT_of(ci, cb),
                                w_sb[("wv", cb)][:, b * 128:(b + 1) * 128],
                                start=(cb == 0), stop=(cb == 1))
                    evac(v[:rows, cig * 1024:(cig + 1) * 1024],
                         ps[:rows, :])

            def pv_pair(pair, tiles):
                for ci in range(nch):
                    for h, (q0, qn) in enumerate(HALVES[b]):
                        ps = p_pvps.tile([128, 288], F32, name=f"pvps{b}",
                                         tag=f"pvps{b}")
                        k = 0
                        nmm = len(tiles)
                        for g in tiles:
                            rows = 64 if (b == 1 and g == 4) else 128
                            nc.tensor.matmul(
                                ps[:, :qn],
                                vt[g][:rows, ci * 128:(ci + 1) * 128],
                                pt[b][g][:rows, q0:q0 + qn],
                                start=(k == 0), stop=(k == nmm - 1))
                            k += 1
                        dst = att_w[:, ci * ntf + q0:ci * ntf + q0 + qn]
                        if pair == 0:
                            evac(dst, ps[:, :qn], bv_sb[:, b:b + 1])
                        else:
                            nc.vector.tensor_add(dst, ps[:, :qn], dst)

            for j in range(T):
                xwt = [p_xb.tile([128, PIX], BF16, name=f"xb{cb}",
                                 tag=f"xb{cb}") for cb in range(2)]
                for cb in range(2):
                    for hh in range(2):
                        nc.sync.dma_start(
                            out=xwt[cb][:, hh * 4608:(hh + 1) * 4608],
                            in_=xw_d[b].ap()[j, cb * 128:(cb + 1) * 128,
                                             hh * 4608:(hh + 1) * 4608])
                for cb in range(2):
                    src = xwt[cb].rearrange("p (ci tok) -> p ci tok",
                                            ci=nch)[:, :, ntf - tsz:ntf]
                    dst = xwtail[cb].rearrange(
                        "p (ci par tok) -> p ci par tok",
                        ci=nch, par=npar)[:, :, j % npar]
                    nc.vector.tensor_copy(dst, src)

                nfull = 4 if b == 0 else 1
                for t in range(nfull):
                    g = gidx0(j, t) if b == 0 else j
                    vbuild(g, lambda ci, cb, _t=t: xwt[cb][
                        :, ci * ntf + _t * 128:ci * ntf + _t * 128 + 128])

                if b == 0 and j in (1, 3):
                    g = 16 + j // 2
                    vbuild(g, lambda ci, cb: xwtail[cb][
                        :, ci * 128:(ci + 1) * 128])
                if b == 1 and j == 3:
                    vbuild(4, lambda ci, cb: xwtail[cb][
                        :, ci * 64:(ci + 1) * 64], rows=64)

                if j == 1:
                    pv_pair(0, ([0, 1, 2, 3, 4, 5, 6, 7, 16] if b == 0
                                else [0, 1]))
                if j == 3:
                    pv_pair(1, ([8, 9, 10, 11, 12, 13, 14, 15, 17]
                                if b == 0 else [2, 3, 4]))

            esB.close()
            if b == 0:
                es_pt0.close()

        # ================= PHASE D: reorder + 3x3 conv + LeakyReLU ========
        p_att = top.enter_context(tc.tile_pool(name="att", bufs=1))
        att = [p_att.tile([128, 98 * 98], BF16, name=f"att{b}",
                          tag=f"att{b}") for b in range(2)]
        with tc.tile_pool(name="wot", bufs=1) as p_wot, \
             tc.tile_pool(name="dout", bufs=3) as p_do, \
             tc.tile_pool(name="dps", bufs=4, space="PSUM") as p_dps:
            for b in range(2):
                att_border_zero(att[b])
                attv = att[b].rearrange("p (h w) -> p h w", h=98)
                for r in range(24):
                    ps = p_dps.tile([128, 384], F32, name="dps", tag="dps")
                    if b == 0:
                        rhs = ap_of(att_ws[b], r * 24,
                                    [[2304, 4], [1, 24], [576, 4]])
                    else:
                        rhs = ap_of(att_ws[b],
                                    (r % 2) * 4 * 1152 + (r // 2) * 12,
                                    [[1152, 4], [1, 12], [144, 8]])
                    nc.tensor.matmul(ps, ident, rhs, start=True, stop=True)
                    evac(attv[:, 1 + r * 4:1 + r * 4 + 4, 1:97],
                         ps.rearrange("p (a c) -> p a c", a=4))
            wot_sb = []
            for cb in range(2):
                t = p_wot.tile([128, 9, C], BF16, name=f"wot{cb}",
                               tag=f"wot{cb}")
                nc.sync.dma_start(
                    out=t,
                    in_=wot.ap()[:, cb * 128:(cb + 1) * 128, :].rearrange(
                        "t i o -> i t o"))
                wot_sb.append(t)
            attv2 = [att[cb].rearrange("p (h w) -> p h w", h=98)
                     for cb in range(2)]
            for coutb in range(2):
                for rg in range(24):
                    ps = p_dps.tile([128, 384], F32, name="dps", tag="dps")
                    k = 0
                    for cb in range(2):
                        for tap in range(9):
                            dy, dx = divmod(tap, 3)
                            rhs = attv2[cb][:, rg * 4 + dy:rg * 4 + dy + 4,
                                            dx:dx + 96]
                            lhsT = wot_sb[cb][:, tap,
                                              coutb * 128:(coutb + 1) * 128]
                            nc.tensor.matmul(ps, lhsT, rhs,
                                             start=(k == 0), stop=(k == 17))
                            k += 1
                    t1 = p_do.tile([128, 384], F32, name="t1", tag="t1")
                    nc.scalar.activation(out=t1, in_=ps, func=Identity,
                                         bias=bo_sb[:, coutb:coutb + 1],
                                         scale=1.0)
                    t2 = p_do.tile([128, 384], F32, name="t2", tag="t2")
                    nc.vector.scalar_tensor_tensor(
                        out=t2, in0=t1, scalar=0.2, in1=t1,
                        op0=mybir.AluOpType.mult,
                        op1=mybir.AluOpType.max)
                    nc.sync.dma_start(
                        out=out.ap()[coutb * 128:(coutb + 1) * 128,
                                     rg * 384:(rg + 1) * 384],
                        in_=t2)
    return nc


_CACHED = {}


def _get_nc():
    if "nc" not in _CACHED:
        nc = bacc.Bacc("TRN2", debug=False, target_bir_lowering=False)
        build(nc)
        nc.compile()
        _CACHED["nc"] = nc
    return _CACHED["nc"]


def _window_major(xf, b):
    """xf [C, 96, 96] -> [C, 9216] with cols ci*ntf + oh*ohb + ow."""
    psz, ohb = PSZ[b], OHB[b]
    z = xf.reshape(C, ohb, psz, ohb, psz)
    z = np.transpose(z, (0, 2, 4, 1, 3))
    return np.ascontiguousarray(z.reshape(C, PIX))


def make_in_maps(x, wq, bq_, wk, bk_, wv, bv_, wo, bo_):
    import ml_dtypes

    bf = ml_dtypes.bfloat16
    shared = {
        "wqt": np.ascontiguousarray(wq.T.astype(bf)),
        "wkt": np.ascontiguousarray(wk.T.astype(bf)),
        "wvt": np.ascontiguousarray(wv.T.astype(bf)),
        "wot": np.ascontiguousarray(
            wo.transpose(2, 3, 1, 0).reshape(9, C, C).astype(bf)),
        "bq": np.ascontiguousarray(bq_.astype(np.float32)),
        "bk": np.ascontiguousarray(bk_.astype(np.float32)),
        "bv": np.ascontiguousarray(bv_.astype(np.float32)),
        "bo": np.ascontiguousarray(bo_.astype(np.float32)),
    }
    x4 = x.reshape(2 * T, C, H, W).astype(np.float32)
    # per (global frame, branch): window-major bf16 [C, PIX]
    xwb = [[_window_major(x4[g], b).astype(ml_dtypes.bfloat16)
            for g in range(2 * T)] for b in range(2)]
    in_maps = []
    for core in range(NCORES):
        v, f = divmod(core, T)
        order = [v * T + f] + [v * T + g for g in range(T) if g != f]
        m = dict(shared)
        for b in range(2):
            m[f"xw{b}"] = np.ascontiguousarray(
                np.stack([xwb[b][g] for g in order]))
        in_maps.append(m)
    return in_maps


def kernel(**inputs):
    from concourse.bass_utils import run_bass_kernel_spmd

    x = np.asarray(inputs["x"], dtype=np.float32)
    in_maps = make_in_maps(
        x, np.asarray(inputs["wq"]), np.asarray(inputs["bq"]),
        np.asarray(inputs["wk"]), np.asarray(inputs["bk"]),
        np.asarray(inputs["wv"]), np.asarray(inputs["bv"]),
        np.asarray(inputs["wo"]), np.asarray(inputs["bo"]))
    nc = _get_nc()
    res = run_bass_kernel_spmd(nc, in_maps, core_ids=list(range(NCORES)))
    outs = [res.results[c]["out"].reshape(C, H, W) for c in range(NCORES)]
    return np.stack(outs).astype(np.float32)
